# revision 19
# baseline (speedup 1.0000x reference)
"""Trainium2 Bass kernel for nn_ConditionalLayer (MoE-style conditional FC).

Reference semantics (N=16384 rows, D=512 features, C=8 conditions):
    out[n] = sum_c relu( (x[n] * [cond_ids[n]==c]) @ W_c + b_c )
           = relu(x[n] @ W_{c*} + b_{c*}) + corr_{c*},   c* = cond_ids[n]
with corr_c = sum_{c' != c} relu(b_{c'}) >= 0 (masked-out rows still
contribute relu(b_c) per the original masked-batch formulation).

Strategy (expert-parallel, 8 cores == 8 conditions):
  - Host routes rows by condition and ships per-core, per-column-chunk
    partition-major bf16 tensors ([P, KT, sz] / [P, FT, sz]) so every DMA
    descriptor is one contiguous multi-KB run per partition (~286 B/ns vs
    ~174 B/ns for the naive [D, cap] layout).
  - Device: yT = max(xT W + (b+corr), corr) on the 128x128 PE.  bf16 halves
    HBM bytes vs fp32 and bf16 LDWEIGHTS overlap MATMUL via the background
    weight buffer (fp32r weight loads serialize at 4 cyc/row).
  - Dummy matmuls on a zeroed scratch tile warm the PE p-state (HAM) while
    the first DMAs fly, so real matmuls run at the 2.4 GHz warm clock.
  - DMA ring latencies measured: Sync/SP HWDGE ~0.8 us to first byte,
    Scalar/Act HWDGE ~3 us, GpSimd SWDGE ~4.5 us.  So the startup-critical
    loads (W, x0, x1, x2) ride Sync in need-order; consts + late x chunks
    ride Scalar; mid-kernel stores ride GpSimd/Sync; the final store is
    split across the two HWDGE rings for a short drain tail.
  - Post-op split: ACT does relu(z+b) for ft 0-1 (DVE adds corr in a cheap
    bf16 4x pass); DVE does fused max(z+(b+corr), corr) for ft 2-3.
"""

import math

import numpy as np

N, D, C = 16384, 512, 8
NCORES = 8
P = 128
KT = D // P  # 4 k-tiles
FT = D // P  # 4 output feature tiles
RTILE = 512  # one PSUM bank of fp32
NWARM = 14  # dummy matmuls to warm the PE while the first DMAs fly

_PROGRAM_CACHE: dict = {}


def _r_tiles(cap: int):
    """Column chunk sizes: 256 head so the PE starts early, 512s after."""
    sizes = []
    rem = cap
    if rem > 256 + RTILE:
        sizes.append(256)
        rem -= 256
    while rem > 0:
        s = min(RTILE, rem)
        sizes.append(s)
        rem -= s
    return sizes


def _build_program(cap: int):
    import concourse.mybir as mybir
    import concourse.tile as tile
    from concourse import bacc

    f32 = mybir.dt.float32
    bf16 = mybir.dt.bfloat16

    nc = bacc.Bacc("TRN2", target_bir_lowering=False, debug=False)

    sizes = _r_tiles(cap)
    nt = len(sizes)
    last = nt - 1

    # per-chunk partition-major tensors
    xts = [
        nc.dram_tensor(f"x_{i}", [P, KT, sz], bf16, kind="ExternalInput")
        for i, sz in enumerate(sizes)
    ]
    yts = [
        nc.dram_tensor(f"y_{i}", [P, FT, sz], bf16, kind="ExternalOutput")
        for i, sz in enumerate(sizes)
    ]
    # host-packed partition-major weights: w2[p, kt, f] = W[kt*128+p, f]
    w2 = nc.dram_tensor("w2", [P, KT, D], bf16, kind="ExternalInput")
    # cst[:, ft] = (b+corr)[ft*128+p]; [:, FT+ft] = corr; [:, 2FT+ft] = b
    cst = nc.dram_tensor("cst", [P, 3 * FT], f32, kind="ExternalInput")

    # loads: the first chunks (and W) must come up fastest -> Sync ring;
    # later chunks ride the slower-to-start Scalar ring.
    n_sync_x = min(3, nt)

    with tile.TileContext(nc) as tc:
        with (
            tc.tile_pool(name="wpool", bufs=1) as wpool,
            tc.tile_pool(name="cpool", bufs=1) as cpool,
            tc.tile_pool(name="xpool", bufs=5) as xpool,
            tc.tile_pool(name="tpool", bufs=3) as tpool,
            tc.tile_pool(name="opool", bufs=3) as opool,
            tc.tile_pool(name="pspool", bufs=8, space="PSUM") as pspool,
        ):
            # --- loads, in need-order across the two HWDGE rings -------
            # Sync ring (~0.8 us to first byte): x0, W kt01, x1, x2.
            # Scalar ring (~1.5-3 us): consts, W kt23, x3, x4.
            w_sb = wpool.tile([P, KT, D], bf16)
            c_sb = cpool.tile([P, 3 * FT], f32)
            x_tiles = []
            for _i in range(nt):
                x_sb = xpool.tile([P, KT, RTILE], bf16, tag="x", name=f"x_sb{_i}")
                x_tiles.append(x_sb)
            nc.sync.dma_start(x_tiles[0][:, :, : sizes[0]], xts[0][:])
            nc.sync.dma_start(w_sb[:, :1], w2[:, :1])
            nc.sync.dma_start(w_sb[:, 1:2], w2[:, 1:2])
            nc.scalar.dma_start(c_sb[:], cst[:])
            nc.scalar.dma_start(w_sb[:, 2:3], w2[:, 2:3])
            nc.scalar.dma_start(w_sb[:, 3:], w2[:, 3:])
            for i, sz in enumerate(sizes):
                if i == 0:
                    continue
                eng = nc.sync if i < n_sync_x else nc.scalar
                eng.dma_start(x_tiles[i][:, :, :sz], xts[i][:])

            # --- PE warmup -----------------------------------------------
            # Varying nonzero data: zero/constant operands don't toggle the
            # datapath, so the HAM activity monitor never upshifts the clock
            # to K=8/8.  int16 iota over [0x3C00, 0x3EFE] bitcast to bf16
            # gives normal values in [0.0078, 0.124] (no denormals/NaNs).
            scratch_i = cpool.tile([P, 256], mybir.dt.int16)
            nc.gpsimd.iota(
                scratch_i[:], [[1, 256]], base=0x3C00, channel_multiplier=2
            )
            scratch = scratch_i[:].bitcast(bf16)
            warm_ps = pspool.tile([P, RTILE], f32, tag="ps")
            for _ in range(NWARM):
                nc.tensor.matmul(
                    warm_ps[:, :256],
                    lhsT=scratch[:, :P],
                    rhs=scratch[:, :256],
                    start=True,
                    stop=True,
                )

            # --- main loop ---------------------------------------------
            for i, sz in enumerate(sizes):
                x_sb = x_tiles[i]
                o_sb = opool.tile([P, FT, RTILE], bf16, tag="o")
                t_sb = tpool.tile([P, 2, RTILE], bf16, tag="t")
                for ft in range(FT):
                    ps = pspool.tile([P, RTILE], f32, tag="ps")
                    for kt in range(KT):
                        nc.tensor.matmul(
                            ps[:, :sz],
                            lhsT=w_sb[:, kt, ft * P : (ft + 1) * P],
                            rhs=x_sb[:, kt, :sz],
                            start=(kt == 0),
                            stop=(kt == KT - 1),
                        )
                    if ft < 2:
                        # ACT: t = relu(z+b); DVE: o = t + corr (bf16 4x)
                        nc.scalar.activation(
                            t_sb[:, ft, :sz],
                            ps[:, :sz],
                            mybir.ActivationFunctionType.Relu,
                            bias=c_sb[:, 2 * FT + ft : 2 * FT + ft + 1],
                        )
                        nc.vector.tensor_scalar_add(
                            o_sb[:, ft, :sz],
                            t_sb[:, ft, :sz],
                            c_sb[:, FT + ft : FT + ft + 1],
                        )
                    else:
                        # relu(z+b)+corr == max(z+(b+corr), corr), corr>=0
                        nc.vector.tensor_scalar(
                            o_sb[:, ft, :sz],
                            ps[:, :sz],
                            c_sb[:, ft : ft + 1],
                            c_sb[:, FT + ft : FT + ft + 1],
                            mybir.AluOpType.add,
                            mybir.AluOpType.max,
                        )
                # --- stores ---------------------------------------------
                if i == last:
                    # split the final store across the two HWDGE rings
                    nc.scalar.dma_start(yts[i][:, :2], o_sb[:, :2, :sz])
                    nc.sync.dma_start(yts[i][:, 2:], o_sb[:, 2:, :sz])
                elif i % 2 == 0:
                    nc.sync.dma_start(yts[i][:], o_sb[:, :, :sz])
                else:
                    nc.gpsimd.dma_start(yts[i][:], o_sb[:, :, :sz])

    nc.compile()
    return nc


def _get_program(cap: int):
    if cap not in _PROGRAM_CACHE:
        _PROGRAM_CACHE[cap] = _build_program(cap)
    return _PROGRAM_CACHE[cap]


def _route(x, cond_ids, W, b):
    """Host-side routing: group rows by condition, build per-core inputs."""
    import ml_dtypes

    bf16 = ml_dtypes.bfloat16

    x = np.ascontiguousarray(np.asarray(x, dtype=np.float32))
    cond_ids = np.asarray(cond_ids, dtype=np.int32)
    W = np.asarray(W, dtype=np.float32)
    b = np.asarray(b, dtype=np.float32)

    counts = np.bincount(cond_ids, minlength=C)
    cap = max(P, math.ceil(counts.max() / 8) * 8)
    sizes = _r_tiles(cap)
    order = np.argsort(cond_ids, kind="stable")
    starts = np.concatenate([[0], np.cumsum(counts)])

    relu_b = np.maximum(b, 0.0)  # [C, D]
    S = relu_b.sum(axis=0)  # [D]

    in_maps = []
    rows_per_core = []
    for c in range(C):
        rows_c = order[starts[c] : starts[c + 1]]
        rows_per_core.append(rows_c)
        xT_c = np.zeros((D, cap), dtype=bf16)
        if len(rows_c):
            xT_c[:, : len(rows_c)] = x[rows_c].T.astype(bf16)
        xk = xT_c.reshape(KT, P, cap)  # [kt, p, r]
        corr_c = S - relu_b[c]  # >= 0 (sum of relus over the other conds)
        cst = np.concatenate(
            [
                (b[c] + corr_c).reshape(FT, P).T,
                corr_c.reshape(FT, P).T,
                b[c].reshape(FT, P).T,
            ],
            axis=1,
        ).astype(np.float32)
        m = {
            "w2": np.ascontiguousarray(
                W[c].astype(bf16).reshape(KT, P, D).transpose(1, 0, 2)
            ),
            "cst": np.ascontiguousarray(cst),
        }
        off = 0
        for i, sz in enumerate(sizes):
            m[f"x_{i}"] = np.ascontiguousarray(
                xk[:, :, off : off + sz].transpose(1, 0, 2)
            )
            off += sz
        in_maps.append(m)
    return in_maps, rows_per_core, cap, sizes


def run(x, cond_ids, W, b, trace: bool = False):
    """Run the kernel; returns (out, BassKernelResults)."""
    try:
        from concourse.bass_utils import run_bass_kernel_spmd
    except ImportError:
        import sys

        sys.path.append("/opt/trn_rl_repo")
        from concourse.bass_utils import run_bass_kernel_spmd

    in_maps, rows_per_core, cap, sizes = _route(x, cond_ids, W, b)
    nc = _get_program(cap)
    res = run_bass_kernel_spmd(
        nc, in_maps, core_ids=list(range(NCORES)), trace=trace
    )

    out = np.empty((len(np.asarray(cond_ids)), D), dtype=np.float32)
    for c in range(C):
        rows_c = rows_per_core[c]
        if not len(rows_c):
            continue
        yT = np.empty((D, cap), dtype=np.float32)
        off = 0
        for i, sz in enumerate(sizes):
            # y_i is [P, FT, sz]; feature f = ft*128 + p
            yT[:, off : off + sz] = (
                res.results[c][f"y_{i}"]
                .transpose(1, 0, 2)
                .reshape(D, sz)
                .astype(np.float32)
            )
            off += sz
        out[rows_c] = yT[:, : len(rows_c)].T
    return out, res


def kernel(x, cond_ids, W, b):
    out, _ = run(x, cond_ids, W, b, trace=False)
    return out
